# revision 1
# baseline (speedup 1.0000x reference)
"""Trainium2 Bass kernel for nn_LoRALayer: out = (x @ A^T) @ B^T * (alpha/rank).

Sharding: data-parallel over tokens (batch*seq = 8192) across 8 cores,
1024 tokens per core; lora_A / lora_B replicated (tiny).

Per-core device program (SPMD, same NEFF on all 8 cores):
  x shard [1024, 4096] f32 arrives token-major. The TensorE contracts over
  the partition dim, so x tiles are transposed on-chip (PE transpose via
  identity matmul) into d-major layout, then:
    h^T [16, T]   = sum_c  A^T_chunk[128d,16].T @ x^T_chunk[128d, T]
    out [T, 512]  = h^T[16,T].T @ B^T[16, 512-chunk]     (x8 chunks)
  SCALING is folded into B^T on the host. All arithmetic fp32.
"""

import numpy as np

IN_F = 4096
OUT_F = 4096
RANK = 16
SCALING = 32.0 / 16.0  # alpha / rank
N_CORES = 8
TOK_TOTAL = 4 * 2048
TOK_PER_CORE = TOK_TOTAL // N_CORES  # 1024
T_TILE = 128
GROUP_T = 256                          # tokens per group (first-mm moving dim)
N_GROUPS = TOK_PER_CORE // GROUP_T     # 4
D_CHUNKS = IN_F // 128                 # 32
OC = 512                               # out-feature chunk (one PSUM bank)
N_OC = OUT_F // OC                     # 8

# matmul input dtype: "f32" (exact) or "f32r" (fast, relaxed internals)
MM_DTYPE = "f32"

_CACHE = {}


def _build_program(mm_dtype=MM_DTYPE):
    import concourse.bass as bass  # noqa: F401
    import concourse.mybir as mybir
    import concourse.tile as tile
    from concourse import bacc
    from concourse.masks import make_identity

    f32 = mybir.dt.float32
    mmdt = mybir.dt.float32 if mm_dtype == "f32" else mybir.dt.float32r

    nc = bacc.Bacc("TRN2", target_bir_lowering=False, debug=False)

    x_d = nc.dram_tensor("x", [TOK_PER_CORE, IN_F], f32, kind="ExternalInput").ap()
    at_d = nc.dram_tensor("at", [128, D_CHUNKS * RANK], mmdt, kind="ExternalInput").ap()
    bt_d = nc.dram_tensor("bt", [RANK, OUT_F], mmdt, kind="ExternalInput").ap()
    out_d = nc.dram_tensor("out", [TOK_PER_CORE, OUT_F], f32, kind="ExternalOutput").ap()

    with tile.TileContext(nc) as tc:
        with (
            tc.tile_pool(name="const", bufs=1) as const_pool,
            tc.tile_pool(name="xn", bufs=3) as xn_pool,
            tc.tile_pool(name="xt", bufs=2) as xt_pool,
            tc.tile_pool(name="osb", bufs=2) as osb_pool,
            tc.tile_pool(name="hsb", bufs=2) as hsb_pool,
            tc.tile_pool(name="tpsum", bufs=4, space="PSUM") as tpsum_pool,
            tc.tile_pool(name="hpsum", bufs=2, space="PSUM") as hpsum_pool,
            tc.tile_pool(name="opsum", bufs=2, space="PSUM") as opsum_pool,
        ):
            ident = const_pool.tile([128, 128], f32, name="ident")
            make_identity(nc, ident)
            at_sb = const_pool.tile([128, D_CHUNKS * RANK], mmdt, name="at_sb")
            nc.sync.dma_start(out=at_sb[:], in_=at_d[:])
            bt_sb = const_pool.tile([RANK, OUT_F], mmdt, name="bt_sb")
            nc.sync.dma_start(out=bt_sb[:], in_=bt_d[:])

            for g in range(N_GROUPS):
                xt = xt_pool.tile(
                    [128, D_CHUNKS, GROUP_T], mmdt, name="xt", tag="xt"
                )
                for tt in range(GROUP_T // T_TILE):
                    t0 = g * GROUP_T + tt * T_TILE
                    xn = xn_pool.tile([T_TILE, IN_F], f32, name="xn", tag="xn")
                    nc.sync.dma_start(out=xn[:], in_=x_d[t0 : t0 + T_TILE, :])
                    for c in range(D_CHUNKS):
                        tp = tpsum_pool.tile([128, T_TILE], f32, name="tp", tag="tp")
                        nc.tensor.transpose(
                            tp[:], xn[:, c * 128 : (c + 1) * 128], ident[:]
                        )
                        nc.vector.tensor_copy(
                            xt[:, c, tt * T_TILE : (tt + 1) * T_TILE], tp[:]
                        )
                hp = hpsum_pool.tile([RANK, GROUP_T], f32, name="hp", tag="hp")
                for c in range(D_CHUNKS):
                    nc.tensor.matmul(
                        hp[:],
                        at_sb[:, c * RANK : (c + 1) * RANK],
                        xt[:, c, :],
                        start=(c == 0),
                        stop=(c == D_CHUNKS - 1),
                    )
                hsb = hsb_pool.tile([RANK, GROUP_T], mmdt, name="hsb", tag="hsb")
                nc.vector.tensor_copy(hsb[:], hp[:])
                for tt in range(GROUP_T // T_TILE):
                    t0 = g * GROUP_T + tt * T_TILE
                    osb = osb_pool.tile([T_TILE, OUT_F], f32, name="osb", tag="osb")
                    for oc in range(N_OC):
                        op = opsum_pool.tile([T_TILE, OC], f32, name="op", tag="op")
                        nc.tensor.matmul(
                            op[:],
                            hsb[:, tt * T_TILE : (tt + 1) * T_TILE],
                            bt_sb[:, oc * OC : (oc + 1) * OC],
                            start=True,
                            stop=True,
                        )
                        nc.scalar.copy(osb[:, oc * OC : (oc + 1) * OC], op[:])
                    nc.sync.dma_start(out=out_d[t0 : t0 + T_TILE, :], in_=osb[:])

    nc.compile()
    return nc


def _get_program(mm_dtype=MM_DTYPE):
    if mm_dtype not in _CACHE:
        _CACHE[mm_dtype] = _build_program(mm_dtype)
    return _CACHE[mm_dtype]


def _host_inputs(x, lora_A, lora_B):
    x2 = np.ascontiguousarray(np.asarray(x, dtype=np.float32).reshape(TOK_TOTAL, IN_F))
    A = np.asarray(lora_A, dtype=np.float32)
    B = np.asarray(lora_B, dtype=np.float32)
    # at[p, c*16 + r] = A[r, c*128 + p]
    at = np.ascontiguousarray(
        A.reshape(RANK, D_CHUNKS, 128).transpose(2, 1, 0).reshape(128, D_CHUNKS * RANK)
    )
    # bt[r, o] = B[o, r] * SCALING
    bt = np.ascontiguousarray((B.T * np.float32(SCALING)))
    in_maps = []
    for c in range(N_CORES):
        shard = np.ascontiguousarray(
            x2[c * TOK_PER_CORE : (c + 1) * TOK_PER_CORE]
        )
        in_maps.append({"x": shard, "at": at, "bt": bt})
    return in_maps


def run(x, lora_A, lora_B, trace=False, mm_dtype=MM_DTYPE):
    """Run on hardware; returns (output, BassKernelResults)."""
    from concourse.bass_utils import run_bass_kernel_spmd

    nc = _get_program(mm_dtype)
    in_maps = _host_inputs(x, lora_A, lora_B)
    res = run_bass_kernel_spmd(nc, in_maps, list(range(N_CORES)), trace=trace)
    shards = [res.results[c]["out"] for c in range(N_CORES)]
    out = np.concatenate(shards, axis=0).reshape(4, 2048, OUT_F)
    return np.ascontiguousarray(out.astype(np.float32)), res


def kernel(x, lora_A, lora_B):
    out, _ = run(x, lora_A, lora_B, trace=False)
    return out


# revision 3
# speedup vs baseline: 1.0705x; 1.0705x over previous
"""Trainium2 Bass kernel for nn_LoRALayer: out = (x @ A^T) @ B^T * (alpha/rank).

Sharding: data-parallel over tokens (batch*seq = 8192) across 8 cores,
1024 tokens per core; lora_A / lora_B replicated (tiny).

Per-core device program (SPMD, same NEFF on all 8 cores):
  x shard [1024, 4096] f32 arrives token-major. The TensorE contracts over
  the partition dim, so x tiles are transposed on-chip (PE transpose via
  identity matmul) into d-major layout, then:
    h^T [16, T]   = sum_c  A^T_chunk[128d,16].T @ x^T_chunk[128d, T]
    out [T, 512]  = h^T[16,T].T @ B^T[16, 512-chunk]     (x8 chunks)
  SCALING is folded into B^T on the host. All arithmetic fp32.
"""

import numpy as np

IN_F = 4096
OUT_F = 4096
RANK = 16
SCALING = 32.0 / 16.0  # alpha / rank
N_CORES = 8
TOK_TOTAL = 4 * 2048
TOK_PER_CORE = TOK_TOTAL // N_CORES  # 1024
T_TILE = 128
GROUP_T = 256                          # tokens per group (first-mm moving dim)
N_GROUPS = TOK_PER_CORE // GROUP_T     # 4
D_CHUNKS = IN_F // 128                 # 32
OC = 512                               # out-feature chunk (one PSUM bank)
N_OC = OUT_F // OC                     # 8

# matmul input dtype: "f32" (exact) or "f32r" (fast, relaxed internals)
MM_DTYPE = "f32r"

_CACHE = {}


def _build_program(mm_dtype=MM_DTYPE):
    import concourse.bass as bass  # noqa: F401
    import concourse.mybir as mybir
    import concourse.tile as tile
    from concourse import bacc
    from concourse.masks import make_identity

    f32 = mybir.dt.float32
    mmdt = mybir.dt.float32 if mm_dtype == "f32" else mybir.dt.float32r

    nc = bacc.Bacc("TRN2", target_bir_lowering=False, debug=False)

    x_d = nc.dram_tensor("x", [TOK_PER_CORE, IN_F], f32, kind="ExternalInput").ap()
    at_d = nc.dram_tensor("at", [128, D_CHUNKS * RANK], mmdt, kind="ExternalInput").ap()
    bt_d = nc.dram_tensor("bt", [RANK, OUT_F], mmdt, kind="ExternalInput").ap()
    out_d = nc.dram_tensor("out", [TOK_PER_CORE, OUT_F], f32, kind="ExternalOutput").ap()

    with tile.TileContext(nc) as tc:
        with (
            tc.tile_pool(name="const", bufs=1) as const_pool,
            tc.tile_pool(name="xn", bufs=3) as xn_pool,
            tc.tile_pool(name="xt", bufs=2) as xt_pool,
            tc.tile_pool(name="osb", bufs=2) as osb_pool,
            tc.tile_pool(name="hsb", bufs=2) as hsb_pool,
            tc.tile_pool(name="tpsum", bufs=4, space="PSUM") as tpsum_pool,
            tc.tile_pool(name="hpsum", bufs=2, space="PSUM") as hpsum_pool,
            tc.tile_pool(name="opsum", bufs=2, space="PSUM") as opsum_pool,
        ):
            ident = const_pool.tile([128, 128], f32, name="ident")
            make_identity(nc, ident)
            at_sb = const_pool.tile([128, D_CHUNKS * RANK], mmdt, name="at_sb")
            nc.sync.dma_start(out=at_sb[:], in_=at_d[:])
            bt_sb = const_pool.tile([RANK, OUT_F], mmdt, name="bt_sb")
            nc.sync.dma_start(out=bt_sb[:], in_=bt_d[:])

            for g in range(N_GROUPS):
                xt = xt_pool.tile(
                    [128, D_CHUNKS, GROUP_T], mmdt, name="xt", tag="xt"
                )
                xns = []
                for tt in range(GROUP_T // T_TILE):
                    t0 = g * GROUP_T + tt * T_TILE
                    xn = xn_pool.tile([T_TILE, IN_F], f32, name="xn", tag="xn")
                    nc.sync.dma_start(out=xn[:], in_=x_d[t0 : t0 + T_TILE, :])
                    xns.append(xn)
                hp = hpsum_pool.tile([RANK, GROUP_T], f32, name="hp", tag="hp")

                # Interleave transposes with first-matmul chunks: PE transposes
                # don't count as busy for the HAM clock gate, so a long
                # transpose-only stretch re-throttles the PE to 1.2 GHz.
                # Keeping a real matmul in every HAM window holds 2.4 GHz.
                MM_LAG = 2

                def first_mm(c, hp=hp, xt=xt, at_sb=at_sb):
                    nc.tensor.matmul(
                        hp[:],
                        at_sb[:, c * RANK : (c + 1) * RANK],
                        xt[:, c, :],
                        start=(c == 0),
                        stop=(c == D_CHUNKS - 1),
                    )

                for c in range(D_CHUNKS):
                    tp = tpsum_pool.tile([128, GROUP_T], f32, name="tp", tag="tp")
                    for tt in range(GROUP_T // T_TILE):
                        nc.tensor.transpose(
                            tp[:, tt * T_TILE : (tt + 1) * T_TILE],
                            xns[tt][:, c * 128 : (c + 1) * 128],
                            ident[:],
                        )
                    nc.vector.tensor_copy(xt[:, c, :], tp[:])
                    if c >= MM_LAG:
                        first_mm(c - MM_LAG)
                for c in range(D_CHUNKS - MM_LAG, D_CHUNKS):
                    first_mm(c)
                hsb = hsb_pool.tile([RANK, GROUP_T], mmdt, name="hsb", tag="hsb")
                nc.vector.tensor_copy(hsb[:], hp[:])
                for tt in range(GROUP_T // T_TILE):
                    t0 = g * GROUP_T + tt * T_TILE
                    osb = osb_pool.tile([T_TILE, OUT_F], f32, name="osb", tag="osb")
                    for oc in range(N_OC):
                        op = opsum_pool.tile([T_TILE, OC], f32, name="op", tag="op")
                        nc.tensor.matmul(
                            op[:],
                            hsb[:, tt * T_TILE : (tt + 1) * T_TILE],
                            bt_sb[:, oc * OC : (oc + 1) * OC],
                            start=True,
                            stop=True,
                        )
                        nc.scalar.copy(osb[:, oc * OC : (oc + 1) * OC], op[:])
                    nc.sync.dma_start(out=out_d[t0 : t0 + T_TILE, :], in_=osb[:])

    nc.compile()
    return nc


def _get_program(mm_dtype=MM_DTYPE):
    if mm_dtype not in _CACHE:
        _CACHE[mm_dtype] = _build_program(mm_dtype)
    return _CACHE[mm_dtype]


def _host_inputs(x, lora_A, lora_B):
    x2 = np.ascontiguousarray(np.asarray(x, dtype=np.float32).reshape(TOK_TOTAL, IN_F))
    A = np.asarray(lora_A, dtype=np.float32)
    B = np.asarray(lora_B, dtype=np.float32)
    # at[p, c*16 + r] = A[r, c*128 + p]
    at = np.ascontiguousarray(
        A.reshape(RANK, D_CHUNKS, 128).transpose(2, 1, 0).reshape(128, D_CHUNKS * RANK)
    )
    # bt[r, o] = B[o, r] * SCALING
    bt = np.ascontiguousarray((B.T * np.float32(SCALING)))
    in_maps = []
    for c in range(N_CORES):
        shard = np.ascontiguousarray(
            x2[c * TOK_PER_CORE : (c + 1) * TOK_PER_CORE]
        )
        in_maps.append({"x": shard, "at": at, "bt": bt})
    return in_maps


def run(x, lora_A, lora_B, trace=False, mm_dtype=MM_DTYPE):
    """Run on hardware; returns (output, BassKernelResults)."""
    from concourse.bass_utils import run_bass_kernel_spmd

    nc = _get_program(mm_dtype)
    in_maps = _host_inputs(x, lora_A, lora_B)
    res = run_bass_kernel_spmd(nc, in_maps, list(range(N_CORES)), trace=trace)
    shards = [res.results[c]["out"] for c in range(N_CORES)]
    out = np.concatenate(shards, axis=0).reshape(4, 2048, OUT_F)
    return np.ascontiguousarray(out.astype(np.float32)), res


def kernel(x, lora_A, lora_B):
    out, _ = run(x, lora_A, lora_B, trace=False)
    return out
